# revision 11
# baseline (speedup 1.0000x reference)
"""NeighborConsistencyLoss on 8 Trainium2 NeuronCores.

Math:  loss = mean_s(1 - mean_k cos(z[s], z[knn[s,k]]))
            = 1 - (1/(S*K)) * sum_{s,k} u(z[s]) . u(z[knn[s,k]])
where u(x) = x/|x| (eps in max(|a||b|, eps) never binds for randn data).

Sharding: replicate z, shard the S=1000 sampled centers across 8 cores
(125 each, padded to 128). Each core gathers its 128 center rows plus
128*32 = 4096 neighbor rows (2KB each) from z in HBM via indirect
(gather) DMA, normalizes rows, computes the per-core partial sum of
cosines, and writes one scalar. Host combines: loss = 1 - total/(S*K).

Per-core device pipeline:
 - idx[128, 33] int32: col t<32 holds neighbor-tile row indices
   (tile t covers centers 4t..4t+3; partition p -> center 4t+p//32,
   neighbor p%32), col 32 holds the center row indices.
 - 33 indirect DMAs gather one [128, 512] tile each (the HW DGE allows
   one row offset per partition per instruction; a 3-dim out AP lowers
   to ~20x-slower descriptors, so outs are strictly 2-dim).
 - Per tile: row sumsq in one pass (ScalarE Square+accum_out or VectorE
   x*x+accum_out, alternating), rnorm = 1/sqrt(ssq) batched per 4 tiles
   (ACT sqrt + DVE reciprocal), u = x*rnorm cast to bf16 (DVE).
 - Group-sum of normalized neighbors on PE in bf16: lhsT is a sliding
   [128,128] slice of a constant 0/1 block mask (lhsT[p,m]=1 iff
   m==4t+p//32), 32 accumulating matmuls -> V[s,:] = sum_k u(n_sk),
   fp32 in PSUM.
 - r[s] = rnorm_c[s] * sum_d c[s,d]*V[s,d] (fused DVE op with accum),
   partial = mask^T @ r via a tiny matmul (mask zeroes the 3 pad slots).
"""

import numpy as np

N, D, K, S = 200000, 512, 32, 1000
NCORES = 8
SPC = S // NCORES            # 125 samples per core
P = 128                      # padded per-core center count
NT = 32                      # neighbor tiles per core (4 centers x 32 rows)
GRP = 4                      # tiles per rnorm batch

_cache = {}


def _build_module():
    import concourse.bacc as bacc
    import concourse.bass as bass
    import concourse.mybir as mybir
    import concourse.tile as tile

    f32 = mybir.dt.float32
    bf16 = mybir.dt.bfloat16
    i32 = mybir.dt.int32
    AF = mybir.ActivationFunctionType
    ALU = mybir.AluOpType

    nc = bacc.Bacc(None, target_bir_lowering=False)
    z_t = nc.dram_tensor("z", [N, D], f32, kind="ExternalInput")
    idx_t = nc.dram_tensor("idx", [P, NT + 1], i32, kind="ExternalInput")
    w_t = nc.dram_tensor("wmask", [P, 256], bf16, kind="ExternalInput")
    mask_t = nc.dram_tensor("mask", [P, 1], f32, kind="ExternalInput")
    out_t = nc.dram_tensor("out", [1, 1], f32, kind="ExternalOutput")

    with tile.TileContext(nc) as tc:
        with (
            tc.tile_pool(name="const", bufs=1) as const,
            tc.tile_pool(name="gath", bufs=1) as gath,
            tc.tile_pool(name="scr", bufs=2) as scr,
            tc.tile_pool(name="ub", bufs=4) as ub,
            tc.tile_pool(name="ps", bufs=1, space="PSUM") as ps,
        ):
            # split the idx load so the first gathers start sooner
            idx_sb = const.tile([P, NT + 1], i32, tag="idx")
            nc.sync.dma_start(idx_sb[:, 0:GRP], idx_t[:, 0:GRP])
            nc.sync.dma_start(idx_sb[:, GRP:NT + 1], idx_t[:, GRP:NT + 1])
            w_sb = const.tile([P, 256], bf16, tag="wmask")
            nc.sync.dma_start(w_sb[:], w_t[:])
            mask_sb = const.tile([P, 1], f32, tag="mask")
            nc.sync.dma_start(mask_sb[:], mask_t[:])

            # gather the center tile FIRST: its data and normalization are
            # needed by the final reduction, so keeping them off the tail
            # (it used to be the 34th gather) shortens the critical path
            ctile = gath.tile([P, D], f32, tag="ctile")
            nc.gpsimd.indirect_dma_start(
                out=ctile[:],
                out_offset=None,
                in_=z_t[:],
                in_offset=bass.IndirectOffsetOnAxis(
                    ap=idx_sb[:, NT:NT + 1], axis=0
                ),
            )
            chunks = []
            for g in range(NT):
                # NOTE: gather outs MUST be 2-dim [P, D] APs; a 3-dim
                # [P, 1, D] tile lowers to pathological descriptors (~20x
                # slower DMA, measured 1.28ms vs 64us for the same bytes).
                ch = gath.tile([P, D], f32, tag=f"ch{g}")
                nc.gpsimd.indirect_dma_start(
                    out=ch[:],
                    out_offset=None,
                    in_=z_t[:],
                    in_offset=bass.IndirectOffsetOnAxis(
                        ap=idx_sb[:, g:g + 1], axis=0
                    ),
                )
                chunks.append(ch)

            V = ps.tile([P, D], f32, tag="V")

            # normalize the center tile early (program order places these
            # at the front of the ACT/DVE queues, overlapping the gathers)
            ssq_c = const.tile([P, 1], f32, tag="ssqc")
            sqr_c = const.tile([P, 1], f32, tag="sqrc")
            rno_c = const.tile([P, 1], f32, tag="rnoc")
            sc_c = scr.tile([P, D], f32, tag="act_sq")
            nc.scalar.activation(sc_c[:], ctile[:], AF.Square,
                                 accum_out=ssq_c[:])
            nc.scalar.activation(sqr_c[:], ssq_c[:], AF.Sqrt)
            nc.vector.reciprocal(rno_c[:], sqr_c[:])

            for b in range(NT // GRP):
                ssq = const.tile([P, GRP], f32, tag=f"ssq{b}")
                sqr = const.tile([P, GRP], f32, tag=f"sqr{b}")
                rno = const.tile([P, GRP], f32, tag=f"rno{b}")
                for j in range(GRP):
                    t = b * GRP + j
                    src = chunks[t][:]
                    if t % 2 == 0:
                        sc = scr.tile([P, D], f32, tag="act_sq")
                        nc.scalar.activation(
                            sc[:], src, AF.Square, accum_out=ssq[:, j:j + 1]
                        )
                    else:
                        sc = scr.tile([P, D], f32, tag="dve_sq")
                        nc.vector.scalar_tensor_tensor(
                            out=sc[:], in0=src, scalar=1.0, in1=src,
                            op0=ALU.mult, op1=ALU.mult,
                            accum_out=ssq[:, j:j + 1],
                        )
                nc.scalar.activation(sqr[:], ssq[:], AF.Sqrt)
                nc.vector.reciprocal(rno[:], sqr[:])
                for j in range(GRP):
                    t = b * GRP + j
                    u_bf = ub.tile([P, D], bf16, tag="u")
                    # alternate engines: DVE tensor_scalar runs in 2-port
                    # perf mode which locks GpSimd out of its SWDGE rings
                    # and slows the gather cadence - keep half on ScalarE
                    if t % 2 == 0:
                        nc.scalar.activation(
                            u_bf[:], chunks[t][:], AF.Copy,
                            scale=rno[:, j:j + 1],
                        )
                    else:
                        nc.vector.tensor_scalar_mul(
                            u_bf[:], chunks[t][:], rno[:, j:j + 1]
                        )
                    nc.tensor.matmul(
                        out=V[:], lhsT=w_sb[:, 124 - 4 * t:252 - 4 * t],
                        rhs=u_bf[:],
                        start=(t == 0), stop=(t == NT - 1),
                    )

            wscr = scr.tile([P, D], f32, tag="wscr")
            r = const.tile([P, 1], f32, tag="r")
            nc.vector.scalar_tensor_tensor(
                out=wscr[:], in0=ctile[:], scalar=rno_c[:, :1], in1=V[:],
                op0=ALU.mult, op1=ALU.mult, accum_out=r[:],
            )

            res_ps = ps.tile([1, 1], f32, tag="res")
            nc.tensor.matmul(
                out=res_ps[:], lhsT=mask_sb[:], rhs=r[:], start=True, stop=True
            )
            res_sb = const.tile([1, 1], f32, tag="res_sb")
            nc.vector.tensor_copy(res_sb[:], res_ps[:])
            nc.sync.dma_start(out_t[:], res_sb[:])

    nc.compile()
    return nc


def _get_module():
    if "nc" not in _cache:
        _cache["nc"] = _build_module()
    return _cache["nc"]


def _make_in_maps(z, knn_neighbors, sample_indices):
    import ml_dtypes

    z = np.ascontiguousarray(np.asarray(z, dtype=np.float32))
    knn = np.asarray(knn_neighbors).astype(np.int64)
    sample = np.asarray(sample_indices).astype(np.int64).ravel()
    assert z.shape == (N, D) and knn.shape == (N, K) and sample.shape == (S,)

    # sliding-window block mask: w[p, c] = 1 iff c == 124 + p//32, so the
    # [128,128] slice at col offset 124-4t gives the lhsT mask for tile t
    pp = np.arange(P)
    w = np.zeros((P, 256), dtype=ml_dtypes.bfloat16)
    w[pp, 124 + pp // 32] = 1.0
    maskv = (pp < SPC).astype(np.float32).reshape(P, 1)

    in_maps = []
    for c in range(NCORES):
        s_ids = np.zeros(P, dtype=np.int64)
        s_ids[:SPC] = sample[c * SPC:(c + 1) * SPC]
        nb = knn[s_ids]                               # [128, 32]
        idx = np.empty((P, NT + 1), dtype=np.int32)
        for t in range(NT):
            idx[:, t] = nb[4 * t + pp // 32, pp % 32]
        idx[:, NT] = s_ids
        in_maps.append({"z": z, "idx": idx, "wmask": w, "mask": maskv})
    return in_maps


def _combine(results):
    total = sum(float(res["out"][0, 0]) for res in results)
    return np.array(1.0 - total / (S * K), dtype=np.float32)


def kernel(z, knn_neighbors, sample_indices):
    from concourse.bass_utils import run_bass_kernel_spmd

    nc = _get_module()
    in_maps = _make_in_maps(z, knn_neighbors, sample_indices)
    out = run_bass_kernel_spmd(nc, in_maps, core_ids=list(range(NCORES)))
    return _combine(out.results)


def run_profiled(z, knn_neighbors, sample_indices, **kw):
    """Dev helper: same as kernel() but returns (loss, BassKernelResults)
    with trace/profile enabled."""
    from concourse.bass_utils import run_bass_kernel_spmd

    nc = _get_module()
    in_maps = _make_in_maps(z, knn_neighbors, sample_indices)
    out = run_bass_kernel_spmd(
        nc, in_maps, core_ids=list(range(NCORES)), trace=True, **kw
    )
    return _combine(out.results), out
